# revision 1
# baseline (speedup 1.0000x reference)
"""Trainium2 Bass kernel for banded relative-position attention (sparse_attention).

Reference semantics (per batch b):
  content/query/key = 1x1-conv projections of x, split into 4 heads x 64 dims
  dots[h,t,s] = q.k + 0.3 * q . emb[50+t-s]   restricted to |t-s| <= 50
  w = softmax(dots) over the band
  out[h,c,t] = sum_s w * content + 0.3 * sum_s w * emb[50+t-s, c]
  y = Wfc @ out + bfc;  BatchNorm1d (train stats over (B,T)); relu; * scale

Distribution: data-parallel over batch (B=8 -> 8 cores). The only cross-core
coupling is the BatchNorm statistics: a (128,4) f32 AllReduce.

Per-core algorithm, software-pipelined 4 stages deep over the 8 query blocks
(S0..S3 of different blocks execute concurrently; all bounce DMAs are
4-head batched to respect the ~625ns/issue HWDGE rate):
  - 128-row query blocks, 384-wide 128-aligned key/content windows.
  - S0: rel logits in band layout (t,j) via one matmul per head against
    0.3*emb[100-j]^T (float32r), exp'd, and written to a DRAM scratch with
    the stride trick: rows land at stride 385, the readback uses stride 384,
    so each row returns shifted by its row index (band -> window skew).
    Gap positions read as exact 0 from the zero-prefilled scratch, which
    multiplicatively masks out-of-band scores; sequence-edge masks are
    pre-added in band layout for the first/last blocks only.
  - S1: qk scores over the window (float32r, tf32-like at full PE rate),
    exp on ScalarE, P = exp(qk)*exp(rel) on VectorE into a persistent
    zero-padded window tile; P's band is written to a second scratch.
  - S2: P^T chunks (PE transposes of the padded tile) and V (the band of P
    read back via the 385-stride, transposed on PE); PSUM->SBUF eviction
    split across VectorE/ScalarE per head-pair.
  - S3: weighted-content + emb output matmuls accumulate in one PSUM tile
    per head; an appended ones-column in the content tiles makes PSUM row
    64 the softmax denominator Z for free; normalization is a reciprocal +
    gpsimd partition_broadcast + multiply at eviction.
Final: Wfc projection (bf16), per-core BN stats via bn_stats/bn_aggr, the
AllReduce, then one fused relu(y*A + B) activation per channel tile.
"""

import numpy as np
from contextlib import ExitStack

import concourse.bass as bass
import concourse.bacc as bacc
import concourse.tile as tile
from concourse import mybir
from concourse.bass_utils import run_bass_kernel_spmd
from concourse import library_config

F32 = mybir.dt.float32
F32R = mybir.dt.float32r
BF16 = mybir.dt.bfloat16
AF = mybir.ActivationFunctionType
OP = mybir.AluOpType

T = 1024
C = 256
H = 4
D = 64
R = 50
J = 101        # band width
JE = 102       # even-padded rel matmul width (fp32r dst restriction)
JP = 112       # padded band width for V
BT = 128       # query block rows
NB = T // BT   # 8 blocks
W = 384        # window width (128-aligned)
OFF = BT - R   # 78: band offset inside the window
BC = 228       # band-cover width inside the window ([OFF, OFF+BC))
NBUF = 3       # DRAM bounce buffers in flight
SCR = 385 * 128  # scratch elements (stride-385 grid)
BN_EPS = 1e-5

_cached = {}

# debug knobs
STAGE = 3           # 1: projections only, 2: +attention, 3: full
USE_COLLECTIVE = True


def _build(stage=None, use_collective=None):
    stage = STAGE if stage is None else stage
    use_collective = USE_COLLECTIVE if use_collective is None else use_collective

    nc = bacc.Bacc("TRN2", target_bir_lowering=False, num_devices=8)

    x_d = nc.declare_dram_parameter("x", [C, T], F32, isOutput=False)
    wqt_d = nc.declare_dram_parameter("wqt", [C, C], F32, isOutput=False)
    wkt_d = nc.declare_dram_parameter("wkt", [C, C], F32, isOutput=False)
    wct_d = nc.declare_dram_parameter("wct", [C, C], F32, isOutput=False)
    wft_d = nc.declare_dram_parameter("wft", [C, C], F32, isOutput=False)
    bvec_d = nc.declare_dram_parameter("bvec", [C, 5], F32, isOutput=False)
    bcr_d = nc.declare_dram_parameter("bcr", [1, C], F32, isOutput=False)
    embt_d = nc.declare_dram_parameter("embt", [128, JE], F32, isOutput=False)
    embr_d = nc.declare_dram_parameter("embr", [128, 65], F32, isOutput=False)
    ident_d = nc.declare_dram_parameter("ident", [128, 128], F32, isOutput=False)
    maskf_d = nc.declare_dram_parameter("maskf", [128, J], F32, isOutput=False)
    maskl_d = nc.declare_dram_parameter("maskl", [128, J], F32, isOutput=False)
    out_d = nc.declare_dram_parameter("out", [C, T], F32, isOutput=True)

    with tile.TileContext(nc) as tc, ExitStack() as ctx:
        const = ctx.enter_context(tc.tile_pool(name="const", bufs=1))
        dram = ctx.enter_context(tc.tile_pool(name="dram", bufs=1, space="DRAM"))

        # ---- DRAM bounce scratches -------------------------------------
        rel_scr = [dram.tile([4 * SCR], BF16, tag=f"rel_scr{i}", name=f"rel_scr{i}")
                   for i in range(NBUF)]
        w_scr = [dram.tile([4 * SCR], BF16, tag=f"w_scr{i}", name=f"w_scr{i}")
                 for i in range(NBUF)]

        # ---- constant loads --------------------------------------------
        def load_cast(dparam, shape, dt, tag):
            t = const.tile(shape, dt, tag=tag, name=tag)
            if dt == BF16:
                # SWDGE casts f32->bf16 during the transfer
                nc.gpsimd.dma_start(out=t, in_=dparam[:, :])
            else:
                st = const.tile(shape, F32, tag=tag + "_st", name=tag + "_st")
                nc.sync.dma_start(out=st, in_=dparam[:, :])
                nc.vector.tensor_copy(t, st)
            return t

        embt_r = load_cast(embt_d, [128, JE], F32R, "embt")
        embr_b = load_cast(embr_d, [128, 65], BF16, "embr")
        ident_b = load_cast(ident_d, [128, 128], BF16, "ident")

        wq_r, wk_r, wc_r, wf_b = [], [], [], []
        for kt in range(2):
            sl = slice(kt * 128, kt * 128 + 128)
            wq_r.append(load_cast(wqt_d[sl, :], [128, C], F32R, f"wq{kt}"))
            wk_r.append(load_cast(wkt_d[sl, :], [128, C], F32R, f"wk{kt}"))
            wc_r.append(load_cast(wct_d[sl, :], [128, C], F32R, f"wc{kt}"))
            wf_b.append(load_cast(wft_d[sl, :], [128, C], BF16, f"wf{kt}"))

        def load_f32(dparam, shape, tag):
            t = const.tile(shape, F32, tag=tag, name=tag)
            nc.gpsimd.dma_start(out=t, in_=dparam)
            return t

        bv_t = [load_f32(bvec_d[k * 128:(k + 1) * 128, :], [128, 5], f"bv{k}") for k in range(2)]
        bq_t = [bv_t[k][:, 0:1] for k in range(2)]
        bk_t = [bv_t[k][:, 1:2] for k in range(2)]
        bfc_t = [bv_t[k][:, 2:3] for k in range(2)]
        gs_t = [bv_t[k][:, 3:4] for k in range(2)]
        bs_t = [bv_t[k][:, 4:5] for k in range(2)]
        maskf_t = load_f32(maskf_d[:, :], [128, J], "maskf")
        maskl_t = load_f32(maskl_d[:, :], [128, J], "maskl")

        # content bias, physically broadcast across partitions via DMA
        bcrb_t = const.tile([128, C], F32, tag="bcrb", name="bcrb")
        nc.gpsimd.dma_start(
            out=bcrb_t,
            in_=bass.AP(tensor=bcr_d[:, :].tensor, offset=0, ap=[[0, 128], [1, C]]),
        )

        # zero-prefill the scratches (gap positions must read as 0.0)
        zero_b = const.tile([128, 4 * 385], BF16, tag="zero_b")
        nc.gpsimd.memset(zero_b, 0.0)
        for i in range(NBUF):
            nc.sync.dma_start(
                out=rel_scr[i].rearrange("(p c) -> p c", c=4 * 385),
                in_=zero_b,
            )
            nc.sync.dma_start(
                out=w_scr[i].rearrange("(p c) -> p c", c=4 * 385),
                in_=zero_b,
            )
        # persistent zero-padded P tiles (bands written each block)
        Pf = [const.tile([128, 4, W], BF16, tag=f"Pf{i}", name=f"Pf{i}")
              for i in range(NBUF)]
        for i in range(NBUF):
            nc.gpsimd.memset(Pf[i], 0.0)
        # gpsimd memsets above run in the default library; the loop uses
        # partition_broadcast from the attn library
        nc.gpsimd.load_library(library_config.attn)

        # ---- persistent activations ------------------------------------
        q_r = [const.tile([128, T], F32R, tag=f"q{i}", name=f"q{i}") for i in range(2)]
        kpad_r = [const.tile([128, 1280], F32R, tag=f"kp{i}", name=f"kp{i}") for i in range(2)]
        for i in range(2):
            nc.gpsimd.memset(kpad_r[i][:, 0:128].bitcast(F32), 0.0)
            nc.gpsimd.memset(kpad_r[i][:, 1152:1280].bitcast(F32), 0.0)
        # content tiles: (128 s, 4 heads x (64 content + 1 spare)); tiles 0
        # and 9 are the zero pads for s<0 / s>=T.
        cT = [const.tile([128, 260], BF16, tag=f"cT{i}", name=f"cT{i}") for i in range(10)]
        for i in range(10):
            nc.gpsimd.memset(cT[i], 0.0)
            # ones column per head: Z row of the output matmul
            nc.gpsimd.memset(cT[i].rearrange("p (h c) -> p h c", c=65)[:, :, 64:65], 1.0)
        attn_b = [const.tile([128, T], BF16, tag=f"attn{i}", name=f"attn{i}") for i in range(2)]
        st_t = [const.tile([128, 2, 6], F32, tag=f"st{i}", name=f"st{i}") for i in range(2)]
        for i in range(2):
            nc.gpsimd.memset(attn_b[i], 0.0)
        y_sb = [const.tile([128, T], F32, tag=f"y{i}", name=f"y{i}") for i in range(2)]

        # ---- projections ------------------------------------------------
        with tc.tile_pool(name="xpool", bufs=1) as xpool, \
             tc.tile_pool(name="pp", bufs=2, space="PSUM") as pp:
            x_r = []
            for kt in range(2):
                xs = xpool.tile([128, T], F32, tag=f"xs{kt}")
                nc.sync.dma_start(out=xs, in_=x_d[kt * 128:(kt + 1) * 128, :])
                xr = xpool.tile([128, T], F32R, tag=f"xr{kt}")
                nc.vector.tensor_copy(xr, xs)
                x_r.append(xr)

            # q, k: (o, t) layout
            for (wt, bias, dest, coff) in (
                (wq_r, bq_t, q_r, 0),
                (wk_r, bk_t, kpad_r, 128),
            ):
                for ot in range(2):
                    for tch in range(2):
                        ps = pp.tile([128, 512], F32, tag="pqk")
                        for kt in range(2):
                            nc.tensor.matmul(
                                ps,
                                wt[kt][:, ot * 128:(ot + 1) * 128],
                                x_r[kt][:, tch * 512:(tch + 1) * 512],
                                start=(kt == 0), stop=(kt == 1),
                            )
                        nc.vector.tensor_scalar_add(
                            out=dest[ot][:, coff + tch * 512: coff + (tch + 1) * 512],
                            in0=ps, scalar1=bias[ot],
                        )

            # content: (s, c) transposed layout
            for st in range(8):
                ps = pp.tile([128, C], F32, tag="pc")
                for kt in range(2):
                    nc.tensor.matmul(
                        ps,
                        x_r[kt][:, st * 128:(st + 1) * 128],
                        wc_r[kt][:, :],
                        start=(kt == 0), stop=(kt == 1),
                    )
                nc.vector.tensor_tensor(
                    out=cT[st + 1].rearrange("p (h c) -> p h c", c=65)[:, :, 0:64],
                    in0=ps.rearrange("p (h c) -> p h c", c=64),
                    in1=bcrb_t[:, :].rearrange("p (h c) -> p h c", c=64),
                    op=OP.add,
                )

        # ---- attention loop (4-head-batched, software-pipelined) ---------
        # Per 128-row block i (all 4 heads together):
        #   S0: rel matmuls + exp + one batched band->DRAM write
        #   S1: one skewed read + qk matmuls + exp + multiply into the
        #       zero-padded window tile + one batched window->DRAM write
        #   S2: one batched band read; per-head PE transposes of P and the
        #       band (V); PSUM->SBUF eviction
        #   S3: output matmuls + 1/Z normalize (gpsimd broadcast) + evict
        if stage >= 2:
            with tc.tile_pool(name="ps_rel", bufs=2, space="PSUM") as ps_rel, \
                 tc.tile_pool(name="ps_s", bufs=1, space="PSUM") as ps_s, \
                 tc.tile_pool(name="ps_t", bufs=1, space="PSUM") as ps_t, \
                 tc.tile_pool(name="ps_o", bufs=2, space="PSUM") as ps_o, \
                 tc.tile_pool(name="ps_y", bufs=1, space="PSUM") as ps_y, \
                 tc.tile_pool(name="sbl", bufs=3) as sbl:
                PT_of = {}
                erw_of = {}
                wb_of = {}

                def q_slice(h, i):
                    ht, hp = h // 2, (h % 2) * 64
                    return q_r[ht][hp:hp + 64, i * BT:(i + 1) * BT]

                def grid_w(scr, width, off):
                    # (p, h, j) view of the 4-head stride-385 write grid
                    v = scr.rearrange("(h x) -> h x", h=4).rearrange(
                        "h (p c) -> h p c", c=385)[:, :, off:off + width]
                    return v.transpose([1, 0, 2])

                def grid_r(scr, width, off):
                    # (p, h, u) view of the 4-head stride-384 read grid
                    v = scr.rearrange("(h x) -> h x", h=4)[:, 0:128 * W].rearrange(
                        "h (p c) -> h p c", c=W)[:, :, off:off + width]
                    return v.transpose([1, 0, 2])

                def S0(i):
                    buf = i % NBUF
                    er_all = sbl.tile([128, 4, J], BF16, tag="er", name=f"er{i}")
                    for h in range(H):
                        pr = ps_rel.tile([128, JE], F32, tag="rel", name=f"pr{i}_{h}")
                        nc.tensor.matmul(pr, q_slice(h, i),
                                         embt_r[(h % 2) * 64:(h % 2) * 64 + 64, :],
                                         start=True, stop=True)
                        if i == 0 or i == NB - 1:
                            msk = maskf_t if i == 0 else maskl_t
                            tmp = sbl.tile([128, J], F32, tag="rtmp", name=f"rt{i}_{h}")
                            nc.vector.tensor_tensor(out=tmp, in0=pr[:, 0:J],
                                                    in1=msk, op=OP.add)
                            nc.scalar.activation(er_all[:, h, :], tmp, AF.Exp)
                        else:
                            nc.scalar.activation(er_all[:, h, :], pr[:, 0:J], AF.Exp)
                    nc.sync.dma_start(out=grid_w(rel_scr[buf], J, OFF), in_=er_all)
                    erw_all = sbl.tile([128, 4, BC], BF16, tag="erw", name=f"erw{i}")
                    nc.sync.dma_start(out=erw_all, in_=grid_r(rel_scr[buf], BC, OFF))
                    erw_of[i] = erw_all

                def S1(i):
                    buf = i % NBUF
                    erw_all = erw_of.pop(i)
                    eqk_all = sbl.tile([128, 4, BC], BF16, tag="eqk", name=f"eqk{i}")
                    for h in range(H):
                        ht, hp = h // 2, (h % 2) * 64
                        pS = ps_s.tile([128, W], F32, tag="S", name=f"pS{i}_{h}")
                        nc.tensor.matmul(pS, q_slice(h, i),
                                         kpad_r[ht][hp:hp + 64, i * BT:i * BT + W],
                                         start=True, stop=True)
                        nc.scalar.activation(eqk_all[:, h, :], pS[:, OFF:OFF + BC], AF.Exp)
                    nc.vector.tensor_tensor(out=Pf[buf][:, :, OFF:OFF + BC],
                                            in0=eqk_all, in1=erw_all, op=OP.mult)
                    nc.sync.dma_start(out=grid_r(w_scr[buf], BC, OFF),
                                      in_=Pf[buf][:, :, OFF:OFF + BC])
                    wband_all = sbl.tile([128, 4, 128], BF16, tag="wband", name=f"wb{i}")
                    nc.sync.dma_start(out=wband_all, in_=grid_w(w_scr[buf], 128, OFF))
                    wb_of[i] = wband_all

                def S2(i):
                    buf = i % NBUF
                    wband_all = wb_of.pop(i)
                    PT = sbl.tile([128, 4, 512], BF16, tag="PT", name=f"PT{i}")
                    for pair in range(2):
                        psT = ps_t.tile([128, 2, 512], BF16, tag=f"T{pair}",
                                        name=f"psT{i}_{pair}")
                        for hh in range(2):
                            h = pair * 2 + hh
                            for ch in range(3):
                                nc.tensor.transpose(psT[:, hh, ch * 128:(ch + 1) * 128],
                                                    Pf[buf][:, h, ch * 128:(ch + 1) * 128],
                                                    ident_b)
                            nc.tensor.transpose(psT[:, hh, 384:512],
                                                wband_all[:, h, :], ident_b)
                        if pair == 0:
                            nc.vector.tensor_copy(PT[:, 0:2, :], psT)
                        else:
                            nc.scalar.activation(PT[:, 2:4, :], psT, AF.Copy)
                    PT_of[i] = PT

                def S3(i):
                    PTa = PT_of.pop(i)
                    t0 = i * BT
                    for h in range(H):
                        ht, hp = h // 2, (h % 2) * 64
                        pO = ps_o.tile([65, BT], F32, tag="O", name=f"pO{i}_{h}")
                        for ch in range(3):
                            nc.tensor.matmul(
                                pO,
                                cT[i + ch][:, 65 * h:65 * h + 65],
                                PTa[:, h, ch * 128:(ch + 1) * 128],
                                start=(ch == 0), stop=False,
                            )
                        nc.tensor.matmul(pO, embr_b[:, :],
                                         PTa[:, h, 384:512],
                                         start=False, stop=True)
                        rzr = sbl.tile([1, BT], F32, tag="rzr", name=f"rzr{i}_{h}")
                        nc.vector.reciprocal(rzr, pO[64:65, :])
                        rzb = sbl.tile([64, BT], F32, tag="rzb", name=f"rzb{i}_{h}")
                        nc.gpsimd.partition_broadcast(rzb, rzr)
                        nc.vector.tensor_tensor(
                            out=attn_b[ht][hp:hp + 64, t0:t0 + BT],
                            in0=pO[0:64, :], in1=rzb, op=OP.mult)

                def fin_chunk(tch):
                    # Wfc projection + per-chunk BN stats for t-columns
                    # [512*tch, 512*tch+512) -- depends only on blocks
                    # 4*tch..4*tch+3, so it overlaps the attention loop.
                    for ot in range(2):
                        py = ps_y.tile([128, 512], F32, tag="y", name=f"py{tch}_{ot}")
                        for kt in range(2):
                            nc.tensor.matmul(
                                py,
                                wf_b[kt][:, ot * 128:(ot + 1) * 128],
                                attn_b[kt][:, tch * 512:(tch + 1) * 512],
                                start=(kt == 0), stop=(kt == 1),
                            )
                        nc.vector.tensor_scalar_add(
                            out=y_sb[ot][:, tch * 512:(tch + 1) * 512],
                            in0=py, scalar1=bfc_t[ot],
                        )
                        nc.vector.bn_stats(
                            out=st_t[ot][:, tch, :],
                            in_=y_sb[ot][:, tch * 512:(tch + 1) * 512])

                for k in range(NB + 3):
                    if k < NB:
                        S0(k)
                    if 0 <= k - 1 < NB:
                        S1(k - 1)
                    if 0 <= k - 2 < NB:
                        S2(k - 2)
                    if 0 <= k - 3 < NB:
                        S3(k - 3)
                    if k - 3 == 3:
                        fin_chunk(0)
                    if k - 3 == NB - 1:
                        fin_chunk(1)

        # ---- BN aggregation + relu*scale (Wfc/stats ran in the loop) -----
        if stage >= 3:
            with tc.tile_pool(name="fin", bufs=1) as fin:
                ccin = fin.tile([128, 4], F32, tag="ccin")
                for ot in range(2):
                    mv = fin.tile([128, nc.vector.BN_AGGR_DIM], F32, tag=f"mv{ot}")
                    nc.vector.bn_aggr(out=mv, in_=st_t[ot])
                    m2 = fin.tile([128, 1], F32, tag=f"m2{ot}")
                    nc.vector.tensor_tensor(out=m2, in0=mv[:, 0:1], in1=mv[:, 0:1], op=OP.mult)
                    nc.vector.tensor_copy(ccin[:, 2 * ot:2 * ot + 1], mv[:, 0:1])
                    nc.vector.tensor_tensor(out=ccin[:, 2 * ot + 1:2 * ot + 2],
                                            in0=mv[:, 1:2], in1=m2, op=OP.add)

                gst = fin.tile([128, 4], F32, tag="gst")
                if use_collective:
                    cc_in = dram.tile([128, 4], F32, tag="cc_in", name="cc_in")
                    cc_out = dram.tile([128, 4], F32, tag="cc_out", name="cc_out")
                    nc.gpsimd.dma_start(out=cc_in[:, :], in_=ccin)
                    nc.gpsimd.collective_compute(
                        "AllReduce", OP.add,
                        replica_groups=[list(range(8))],
                        ins=[cc_in.opt()],
                        outs=[cc_out.opt()],
                    )
                    nc.gpsimd.dma_start(out=gst, in_=cc_out[:, :])
                else:
                    # debug bypass: every core uses 8x its own stats
                    nc.vector.tensor_scalar_mul(out=gst, in0=ccin, scalar1=8.0)

                eps_t = fin.tile([128, 1], F32, tag="eps")
                nc.vector.memset(eps_t, BN_EPS)
                for ot in range(2):
                    mg = fin.tile([128, 1], F32, tag=f"mg{ot}")
                    nc.vector.tensor_scalar_mul(out=mg, in0=gst[:, 2 * ot:2 * ot + 1], scalar1=0.125)
                    eg = fin.tile([128, 1], F32, tag=f"eg{ot}")
                    nc.vector.tensor_scalar_mul(out=eg, in0=gst[:, 2 * ot + 1:2 * ot + 2], scalar1=0.125)
                    mg2 = fin.tile([128, 1], F32, tag=f"mg2{ot}")
                    nc.vector.tensor_tensor(out=mg2, in0=mg, in1=mg, op=OP.mult)
                    var = fin.tile([128, 1], F32, tag=f"var{ot}")
                    nc.vector.tensor_tensor(out=var, in0=eg, in1=mg2, op=OP.subtract)
                    sd = fin.tile([128, 1], F32, tag=f"sd{ot}")
                    nc.scalar.activation(sd, var, AF.Sqrt, bias=eps_t)
                    rstd = fin.tile([128, 1], F32, tag=f"rstd{ot}")
                    nc.vector.reciprocal(rstd, sd)
                    a_t = fin.tile([128, 1], F32, tag=f"a{ot}")
                    nc.vector.tensor_tensor(out=a_t, in0=rstd, in1=gs_t[ot], op=OP.mult)
                    ma = fin.tile([128, 1], F32, tag=f"ma{ot}")
                    nc.vector.tensor_tensor(out=ma, in0=mg, in1=a_t, op=OP.mult)
                    b_t = fin.tile([128, 1], F32, tag=f"b{ot}")
                    nc.vector.tensor_tensor(out=b_t, in0=bs_t[ot], in1=ma, op=OP.subtract)
                    for tch in range(2):
                        yo = fin.tile([128, 512], F32, tag=f"yo{ot}_{tch}",
                                      name=f"yo{ot}_{tch}")
                        nc.scalar.activation(yo, y_sb[ot][:, tch * 512:(tch + 1) * 512],
                                             AF.Relu, bias=b_t, scale=a_t)
                        nc.sync.dma_start(
                            out=out_d[ot * 128:(ot + 1) * 128, tch * 512:(tch + 1) * 512],
                            in_=yo)
        else:
            # debug dump of intermediate state
            with tc.tile_pool(name="dump", bufs=1) as dump:
                for ot in range(2):
                    af = dump.tile([128, T], F32, tag=f"af{ot}", name=f"af{ot}")
                    if stage >= 2:
                        nc.vector.tensor_copy(af, attn_b[ot])
                    else:
                        nc.vector.tensor_copy(af, q_r[ot].bitcast(F32))
                    nc.sync.dma_start(out=out_d[ot * 128:(ot + 1) * 128, :], in_=af)

    nc.compile()
    return nc


def _prep_host(inputs):
    x = np.ascontiguousarray(inputs["x"], dtype=np.float32)
    emb = np.asarray(inputs["emb"], dtype=np.float32)
    er = np.ascontiguousarray(0.3 * emb[::-1])          # (101, 64): er[j] = 0.3*emb[100-j]
    embt = np.zeros((128, JE), dtype=np.float32)
    embt[0:64, :J] = er.T
    embt[64:128, :J] = er.T
    embr = np.zeros((128, 65), dtype=np.float32)
    embr[:J, :64] = er

    p = np.arange(128)[:, None]
    j = np.arange(J)[None, :]
    # block 0: t = p, s = p+j-50 >= 0  ->  valid j >= 50-p
    maskf = np.where(j < R - p, -30000.0, 0.0).astype(np.float32)
    # block 7: t = 896+p, s = 846+p+j <= 1023  ->  valid j <= 177-p
    maskl = np.where(j > 177 - p, -30000.0, 0.0).astype(np.float32)

    shared = {
        "wqt": np.ascontiguousarray(inputs["Wq"].T, dtype=np.float32),
        "wkt": np.ascontiguousarray(inputs["Wk"].T, dtype=np.float32),
        "wct": np.ascontiguousarray(inputs["Wc"].T, dtype=np.float32),
        "wft": np.ascontiguousarray(inputs["Wfc"].T, dtype=np.float32),
        "bvec": np.stack([
            np.asarray(inputs["bq"], np.float32),
            np.asarray(inputs["bk"], np.float32),
            np.asarray(inputs["bfc"], np.float32),
            np.asarray(inputs["gamma"], np.float32) * np.asarray(inputs["scale"], np.float32),
            np.asarray(inputs["beta"], np.float32) * np.asarray(inputs["scale"], np.float32),
        ], axis=1).copy(),
        "bcr": np.asarray(inputs["bc"], np.float32).reshape(1, C).copy(),
        "embt": np.ascontiguousarray(embt),
        "embr": embr,
        "ident": np.eye(128, dtype=np.float32),
        "maskf": maskf,
        "maskl": maskl,
    }
    return x, shared


def kernel(**inputs):
    if "nc" not in _cached:
        _cached["nc"] = _build()
    nc = _cached["nc"]

    x, shared = _prep_host(inputs)
    B = x.shape[0]
    in_maps = [dict(shared, x=np.ascontiguousarray(x[b])) for b in range(B)]
    res = run_bass_kernel_spmd(nc, in_maps, core_ids=list(range(8)))
    out = np.stack([res.results[b]["out"] for b in range(B)], axis=0)
    return out.astype(np.float32)

